# revision 12
# baseline (speedup 1.0000x reference)
"""GNN NodeBlock (MolDiff-style message passing) on 8 Trainium2 NeuronCores.

Contract: kernel(**full inputs) -> full output. Internally:
 - Nodes padded to 50176 = 392 tiles of 128; dest tiles split contiguously
   across 8 cores (49 each) so the segment-sum is core-local.
 - Phase 1 (device): per-node table [h_node | u]; LN first-layer weights are
   host-centered so the LN mean term vanishes exactly.
 - Host: gathers table rows per edge (data movement only) into feature-major
   streams, edges sorted by dest and padded to a shared per-tile chunk layout
   (identical across cores - SPMD requires one program).
 - Phase 2 (device): feature-major MLP chain; variance via squared-activation
   matmuls; final msg/gate matmuls take feature-major activations as lhsT so
   outputs land batch-major without transposes; one-hot S matmul accumulates
   the segment sum in PSUM per dest tile; fused output MLP per tile.
"""
import numpy as np

N, E = 50000, 800000
ND, ED, H = 128, 64, 128
EPS = 1e-5
P = 128
NCORES = 8
NT = 392
NPAD = NT * P
TILES_PC = NT // NCORES
B = 512
CPB = B // P

RSTD_FALLBACK = False  # True -> Sqrt+reciprocal instead of Abs_reciprocal_sqrt


def _to_np(v):
    return np.asarray(v, dtype=np.float32)


def _center(w):
    return w - w.mean(axis=1, keepdims=True)


def _bf16(a):
    import ml_dtypes
    return np.ascontiguousarray(a).astype(ml_dtypes.bfloat16)


def _trivial(g, b):
    return bool(np.all(g == 1.0) and np.all(b == 0.0))


def _build_phase1(affine_triv):
    import concourse.bacc as bacc
    import concourse.tile as tile
    from concourse import mybir
    from concourse.masks import make_identity
    from contextlib import ExitStack

    BF16 = mybir.dt.bfloat16
    F32 = mybir.dt.float32
    AF = mybir.ActivationFunctionType
    OP = mybir.AluOpType
    NTC = TILES_PC

    nc = bacc.Bacc("TRN2", target_bir_lowering=False, debug=False)
    xT = nc.declare_dram_parameter("xT", [ND, NTC * P], BF16, isOutput=False)
    n1T = nc.declare_dram_parameter("n1T", [2, NTC * P], BF16, isOutput=False)
    wts = nc.declare_dram_parameter("wts", [P, 5 * P], BF16, isOutput=False)
    brows = nc.declare_dram_parameter("brows", [1, 3 * P], BF16, isOutput=False)
    tbl = nc.declare_dram_parameter("tbl", [NTC * P, 2 * H], BF16, isOutput=True)

    with tile.TileContext(nc) as tc, ExitStack() as ctx:
        pool = ctx.enter_context(tc.tile_pool(name="sb", bufs=2))
        cpool = ctx.enter_context(tc.tile_pool(name="const", bufs=1))
        psZ = ctx.enter_context(tc.tile_pool(name="psZ", bufs=2, space="PSUM"))
        psH = ctx.enter_context(tc.tile_pool(name="psH", bufs=2, space="PSUM"))
        psN = ctx.enter_context(tc.tile_pool(name="psN", bufs=2, space="PSUM"))
        psU = ctx.enter_context(tc.tile_pool(name="psU", bufs=2, space="PSUM"))

        w_t = cpool.tile([P, 5 * P], BF16)
        nc.sync.dma_start(out=w_t[:], in_=wts[:, :])
        W1n = w_t[:, 0:P]
        W2n = w_t[:, P:2 * P]
        Wxg = w_t[:, 2 * P:3 * P]
        gbc = w_t[:, 3 * P:5 * P]    # [g1 bcast | be1 bcast]
        br_t = cpool.tile([1, 3 * P], BF16)
        nc.sync.dma_start(out=br_t[:], in_=brows[:, :])
        b1n_r = br_t[:, 0:P]
        b2n_r = br_t[:, P:2 * P]
        wtg_r = br_t[:, 2 * P:3 * P]

        xT_t = cpool.tile([ND, NTC * P], BF16)
        nc.sync.dma_start(out=xT_t[:], in_=xT[:, :])
        nt_t = cpool.tile([1, NTC * P], BF16)
        nc.sync.dma_start(out=nt_t[:], in_=n1T[0:1, :])
        on_t = cpool.tile([1, NTC * P], BF16)
        nc.sync.dma_start(out=on_t[:], in_=n1T[1:2, :])
        ident = cpool.tile([P, P], BF16)
        make_identity(nc, ident[:])
        eps_t = cpool.tile([P, 1], F32)
        nc.vector.memset(eps_t[:], EPS)

        for t in range(NTC):
            cs = slice(t * P, (t + 1) * P)
            z1 = psZ.tile([P, P], F32, space="PSUM", tag="z")
            nc.tensor.matmul(out=z1[:], lhsT=xT_t[:, cs], rhs=W1n[:, :],
                             start=True, stop=False)
            nc.tensor.matmul(out=z1[:], lhsT=on_t[:, cs], rhs=b1n_r,
                             start=False, stop=True)
            mv_s = pool.tile([P, 6], F32, tag="mvs")
            nc.vector.bn_stats(out=mv_s[:], in_=z1[:])
            mv = pool.tile([P, 2], F32, tag="mv")
            nc.vector.bn_aggr(out=mv[:], in_=mv_s[:])
            rstd = pool.tile([P, 1], F32, tag="rstd")
            nc.scalar.activation(out=rstd[:], in_=mv[:, 1:2], func=AF.Sqrt,
                                 bias=eps_t[:, :1], scale=1.0)
            nc.vector.reciprocal(out=rstd[:], in_=rstd[:])
            h1 = pool.tile([P, P], BF16, tag="h1")
            if affine_triv:
                nc.scalar.activation(out=h1[:], in_=z1[:], func=AF.Relu,
                                     bias=0.0, scale=rstd[:, :1])
            else:
                tmp = pool.tile([P, P], F32, tag="tmp")
                nc.vector.tensor_scalar(out=tmp[:], in0=z1[:],
                                        scalar1=rstd[:, :1], scalar2=None,
                                        op0=OP.mult)
                nc.vector.tensor_tensor(out=tmp[:], in0=tmp[:],
                                        in1=gbc[:, 0:P], op=OP.mult)
                nc.vector.tensor_tensor(out=tmp[:], in0=tmp[:],
                                        in1=gbc[:, P:2 * P], op=OP.add)
                nc.scalar.activation(out=h1[:], in_=tmp[:], func=AF.Relu,
                                     bias=0.0, scale=1.0)
            h1T_p = psH.tile([P, P], BF16, space="PSUM", tag="h1T")
            nc.tensor.transpose(out=h1T_p[:], in_=h1[:], identity=ident[:])
            h1T = pool.tile([P, P], BF16, tag="h1Ts")
            nc.vector.tensor_copy(out=h1T[:], in_=h1T_p[:])
            hn = psN.tile([P, P], F32, space="PSUM", tag="hn")
            nc.tensor.matmul(out=hn[:], lhsT=h1T[:], rhs=W2n[:, :],
                             start=True, stop=False)
            nc.tensor.matmul(out=hn[:], lhsT=on_t[:, cs], rhs=b2n_r,
                             start=False, stop=True)
            u = psU.tile([P, P], F32, space="PSUM", tag="u")
            nc.tensor.matmul(out=u[:], lhsT=xT_t[:, cs], rhs=Wxg[:, :],
                             start=True, stop=False)
            nc.tensor.matmul(out=u[:], lhsT=nt_t[:, cs], rhs=wtg_r,
                             start=False, stop=True)
            comb = pool.tile([P, 2 * H], BF16, tag="comb")
            nc.scalar.activation(out=comb[:, 0:H], in_=hn[:], func=AF.Copy,
                                 bias=0.0, scale=1.0)
            nc.vector.tensor_copy(out=comb[:, H:2 * H], in_=u[:])
            nc.sync.dma_start(out=tbl[cs, :], in_=comb[:])
    nc.compile()
    return nc


def _build_phase2(cmax, tile_chunks, affine_triv_e, affine_triv_g,
                  affine_triv_o, biases_triv):
    import concourse.bacc as bacc
    import concourse.tile as tile
    from concourse import mybir
    from concourse.masks import make_identity
    from contextlib import ExitStack

    BF16 = mybir.dt.bfloat16
    F32 = mybir.dt.float32
    AF = mybir.ActivationFunctionType
    OP = mybir.AluOpType
    NSC = cmax // CPB

    nc = bacc.Bacc("TRN2", target_bir_lowering=False, debug=False)
    eT = nc.declare_dram_parameter("eT", [ED + 1, cmax * P], BF16, isOutput=False)
    hnuT = nc.declare_dram_parameter("hnuT", [2 * H, cmax * P], BF16,
                                     isOutput=False)
    dcols = nc.declare_dram_parameter("dcols", [P, cmax], F32, isOutput=False)
    xT2 = nc.declare_dram_parameter("xT2", [ND, TILES_PC * P], BF16,
                                    isOutput=False)
    wA = nc.declare_dram_parameter("wA", [ED + 1, 2 * P], BF16, isOutput=False)
    wB = nc.declare_dram_parameter("wB", [P, 5 * P], BF16, isOutput=False)
    miscr = nc.declare_dram_parameter("miscr", [1, 6 * P], BF16, isOutput=False)
    bcol = nc.declare_dram_parameter("bcol", [P, 4], F32, isOutput=False)
    gcolg = nc.declare_dram_parameter("gcolg", [P, 2], F32, isOutput=False)
    out = nc.declare_dram_parameter("out", [TILES_PC * P, ND], F32,
                                    isOutput=True)

    with tile.TileContext(nc) as tc, ExitStack() as ctx:
        cpool = ctx.enter_context(tc.tile_pool(name="const", bufs=1))
        epool = ctx.enter_context(tc.tile_pool(name="estream", bufs=3))
        hpool = ctx.enter_context(tc.tile_pool(name="hstream", bufs=3))
        wpool = ctx.enter_context(tc.tile_pool(name="work", bufs=2))
        spool = ctx.enter_context(tc.tile_pool(name="small", bufs=2))
        mpool = ctx.enter_context(tc.tile_pool(name="msgs", bufs=8))
        psZ = ctx.enter_context(tc.tile_pool(name="psZ", bufs=5, space="PSUM"))
        psG = ctx.enter_context(tc.tile_pool(name="psG", bufs=2, space="PSUM"))
        psT = ctx.enter_context(tc.tile_pool(name="psT", bufs=1, space="PSUM"))

        wA_t = cpool.tile([ED + 1, 2 * P], BF16)
        nc.sync.dma_start(out=wA_t[:], in_=wA[:, :])
        W1e = wA_t[:, 0:P]
        W1g = wA_t[:, P:2 * P]
        wB_t = cpool.tile([P, 5 * P], BF16)
        nc.sync.dma_start(out=wB_t[:], in_=wB[:, :])
        W2e = wB_t[:, 0:P]
        W2g = wB_t[:, P:2 * P]
        msgW = wB_t[:, 2 * P:3 * P]
        cenW = wB_t[:, 3 * P:4 * P]
        outW = wB_t[:, 4 * P:5 * P]
        mi_t = cpool.tile([1, 6 * P], BF16)
        nc.sync.dma_start(out=mi_t[:], in_=miscr[:, :])
        msgb_r = mi_t[:, 0:P]
        b2g_r = mi_t[:, P:2 * P]
        cenb_r = mi_t[:, 2 * P:3 * P]
        outb_r = mi_t[:, 3 * P:4 * P]
        lng_r = mi_t[:, 4 * P:5 * P]
        lnb_r = mi_t[:, 5 * P:6 * P]
        bcol_t = cpool.tile([P, 4], F32)
        nc.sync.dma_start(out=bcol_t[:], in_=bcol[:, :])
        gcolg_t = cpool.tile([P, 2], F32)
        nc.sync.dma_start(out=gcolg_t[:], in_=gcolg[:, :])
        dcols_t = cpool.tile([P, cmax], F32)
        nc.sync.dma_start(out=dcols_t[:], in_=dcols[:, :])
        xT2_t = cpool.tile([ND, TILES_PC * P], BF16)
        nc.sync.dma_start(out=xT2_t[:], in_=xT2[:, :])

        ident_b = cpool.tile([P, P], BF16)
        make_identity(nc, ident_b[:])
        iota_i = cpool.tile([P, P], mybir.dt.int32)
        iota_b = cpool.tile([P, P], BF16)
        nc.gpsimd.iota(iota_i[:], pattern=[[1, P]], base=0,
                       channel_multiplier=0)
        nc.vector.tensor_copy(out=iota_b[:], in_=iota_i[:])
        ones128 = cpool.tile([P, P], BF16)
        nc.vector.memset(ones128[:], 1.0)
        ones1 = cpool.tile([1, P], BF16)
        nc.vector.memset(ones1[:], 1.0)
        eps_t = cpool.tile([P, 1], F32)
        nc.vector.memset(eps_t[:], EPS)

        chunk_tile = []
        for t, cnt in enumerate(tile_chunks):
            chunk_tile += [t] * cnt
        assert len(chunk_tile) == cmax
        tile_first = {}
        tile_last = {}
        for ci, t in enumerate(chunk_tile):
            tile_first.setdefault(t, ci)
            tile_last[t] = ci

        aggr_tiles = {}

        def out_stage(t):
            aggr = aggr_tiles.pop(t)
            cs = slice(t * P, (t + 1) * P)
            nc.tensor.matmul(out=aggr[:], lhsT=xT2_t[:, cs], rhs=cenW[:, :],
                             start=False, stop=False)
            nc.tensor.matmul(out=aggr[:], lhsT=ones1[:, :], rhs=cenb_r,
                             start=False, stop=True)
            mv_s = spool.tile([P, 6], F32, tag="omvs")
            nc.vector.bn_stats(out=mv_s[:], in_=aggr[:])
            mv = spool.tile([P, 2], F32, tag="omv")
            nc.vector.bn_aggr(out=mv[:], in_=mv_s[:])
            rstd = spool.tile([P, 1], F32, tag="orstd")
            nc.scalar.activation(out=rstd[:], in_=mv[:, 1:2], func=AF.Sqrt,
                                 bias=eps_t[:, :1], scale=1.0)
            nc.vector.reciprocal(out=rstd[:], in_=rstd[:])
            nmr = spool.tile([P, 1], F32, tag="onmr")
            nc.vector.tensor_scalar(out=nmr[:], in0=mv[:, 0:1],
                                    scalar1=rstd[:, :1], scalar2=-1.0,
                                    op0=OP.mult, op1=OP.mult)
            ho = spool.tile([P, P], BF16, tag="oho")
            if affine_triv_o:
                nc.scalar.activation(out=ho[:], in_=aggr[:], func=AF.Relu,
                                     bias=nmr[:, :1], scale=rstd[:, :1])
            else:
                tmp = spool.tile([P, P], F32, tag="otmp")
                nc.vector.tensor_scalar(out=tmp[:], in0=aggr[:],
                                        scalar1=mv[:, 0:1],
                                        scalar2=rstd[:, :1],
                                        op0=OP.subtract, op1=OP.mult)
                gb_p = psZ.tile([P, P], F32, space="PSUM", tag="pz")
                nc.tensor.matmul(out=gb_p[:], lhsT=ones1[:, :],
                                 rhs=lng_r, start=True, stop=True)
                nc.vector.tensor_tensor(out=tmp[:], in0=tmp[:], in1=gb_p[:],
                                        op=OP.mult)
                gb_p2 = psZ.tile([P, P], F32, space="PSUM", tag="pz")
                nc.tensor.matmul(out=gb_p2[:], lhsT=ones1[:, :],
                                 rhs=lnb_r, start=True, stop=True)
                nc.vector.tensor_tensor(out=tmp[:], in0=tmp[:], in1=gb_p2[:],
                                        op=OP.add)
                nc.scalar.activation(out=ho[:], in_=tmp[:], func=AF.Relu,
                                     bias=0.0, scale=1.0)
            hoT_p = psT.tile([P, P], BF16, space="PSUM", tag="ohoT")
            nc.tensor.transpose(out=hoT_p[:], in_=ho[:], identity=ident_b[:])
            hoT = spool.tile([P, P], BF16, tag="ohoTs")
            nc.vector.tensor_copy(out=hoT[:], in_=hoT_p[:])
            o_p = psZ.tile([P, ND], F32, space="PSUM", tag="pz")
            nc.tensor.matmul(out=o_p[:], lhsT=hoT[:], rhs=outW[:, :],
                             start=True, stop=False)
            nc.tensor.matmul(out=o_p[:], lhsT=ones1[:, :], rhs=outb_r,
                             start=False, stop=True)
            o_s = spool.tile([P, ND], F32, tag="oos")
            nc.scalar.activation(out=o_s[:], in_=o_p[:], func=AF.Copy,
                                 bias=0.0, scale=1.0)
            nc.sync.dma_start(out=out[cs, :], in_=o_s[:])

        for sc in range(NSC):
            e0 = sc * B
            csl = slice(e0, e0 + B)
            eT_t = epool.tile([ED + 1, B], BF16, tag="eT")
            nc.sync.dma_start(out=eT_t[:], in_=eT[:, csl])
            hnu = hpool.tile([P, 2, B], BF16, tag="hnu")
            nc.sync.dma_start(out=hnu[:],
                              in_=hnuT[:, csl].rearrange("(j p) b -> p j b",
                                                         p=P))
            hnT = hnu[:, 0, :]
            uT = hnu[:, 1, :]

            z1 = psZ.tile([P, B], F32, space="PSUM", tag="pz")
            nc.tensor.matmul(out=z1[:], lhsT=W1e[:, :], rhs=eT_t[:],
                             start=True, stop=True)
            zg = psZ.tile([P, B], F32, space="PSUM", tag="pz")
            nc.tensor.matmul(out=zg[:], lhsT=W1g[:, :], rhs=eT_t[:],
                             start=True, stop=False)
            nc.tensor.matmul(out=zg[:], lhsT=ident_b[:], rhs=uT,
                             start=False, stop=True)

            z1sq = wpool.tile([P, B], BF16, tag="z1sq")
            nc.scalar.activation(out=z1sq[:], in_=z1[:], func=AF.Square,
                                 bias=0.0, scale=1.0)
            zgsq = wpool.tile([P, B], BF16, tag="zgsq")
            nc.scalar.activation(out=zgsq[:], in_=zg[:], func=AF.Square,
                                 bias=0.0, scale=1.0)

            ss1b = psZ.tile([P, B], F32, space="PSUM", tag="pz")
            nc.tensor.matmul(out=ss1b[:], lhsT=ones128[:], rhs=z1sq[:],
                             start=True, stop=True)
            ssgb = psZ.tile([P, B], F32, space="PSUM", tag="pz")
            nc.tensor.matmul(out=ssgb[:], lhsT=ones128[:], rhs=zgsq[:],
                             start=True, stop=True)

            rstd1b = wpool.tile([P, B], BF16, tag="rstd1b")
            rstdgb = wpool.tile([P, B], BF16, tag="rstdgb")
            nc.scalar.activation(out=rstd1b[:], in_=ss1b[:],
                                 func=AF.Abs_reciprocal_sqrt,
                                 bias=eps_t[:, :1], scale=1.0 / H)
            nc.scalar.activation(out=rstdgb[:], in_=ssgb[:],
                                 func=AF.Abs_reciprocal_sqrt,
                                 bias=eps_t[:, :1], scale=1.0 / H)
            h1s = wpool.tile([P, B], BF16, tag="h1s")
            if affine_triv_e:
                h1r = wpool.tile([P, B], BF16, tag="h1r")
                nc.scalar.activation(out=h1r[:], in_=z1[:], func=AF.Relu,
                                     bias=0.0, scale=1.0)
                nc.vector.tensor_tensor(out=h1s[:], in0=h1r[:], in1=rstd1b[:],
                                        op=OP.mult)
            else:
                etmp = wpool.tile([P, B], F32, tag="etmp")
                nc.vector.scalar_tensor_tensor(
                    out=etmp[:], in0=z1[:], scalar=bcol_t[:, 1:2],
                    in1=rstd1b[:], op0=OP.mult, op1=OP.mult)
                nc.scalar.activation(out=h1s[:], in_=etmp[:], func=AF.Relu,
                                     bias=bcol_t[:, 2:3], scale=1.0)

            g1s = wpool.tile([P, B], BF16, tag="g1s")
            if affine_triv_g:
                g1r = wpool.tile([P, B], BF16, tag="g1r")
                nc.scalar.activation(out=g1r[:], in_=zg[:], func=AF.Relu,
                                     bias=0.0, scale=1.0)
                nc.vector.tensor_tensor(out=g1s[:], in0=g1r[:], in1=rstdgb[:],
                                        op=OP.mult)
            else:
                gtmp = wpool.tile([P, B], F32, tag="gtmp")
                nc.vector.scalar_tensor_tensor(
                    out=gtmp[:], in0=zg[:], scalar=gcolg_t[:, 0:1],
                    in1=rstdgb[:], op0=OP.mult, op1=OP.mult)
                nc.scalar.activation(out=g1s[:], in_=gtmp[:], func=AF.Relu,
                                     bias=gcolg_t[:, 1:2], scale=1.0)

            zL2 = psZ.tile([P, B], F32, space="PSUM", tag="pz")
            nc.tensor.matmul(out=zL2[:], lhsT=W2e[:], rhs=h1s[:],
                             start=True, stop=True)
            mT = wpool.tile([P, B], BF16, tag="mT")
            nc.vector.scalar_tensor_tensor(
                out=mT[:], in0=zL2[:], scalar=bcol_t[:, 0:1], in1=hnT,
                op0=OP.add, op1=OP.mult)

            gateB = psZ.tile([P, B], F32, space="PSUM", tag="pz")
            msg0B = psZ.tile([P, B], F32, space="PSUM", tag="pz")
            for t in range(CPB):
                osl = slice(t * P, (t + 1) * P)
                nc.tensor.matmul(out=gateB[:, osl], lhsT=g1s[:, osl],
                                 rhs=W2g[:], start=True, stop=biases_triv)
                if not biases_triv:
                    nc.tensor.matmul(out=gateB[:, osl], lhsT=ones1[:],
                                     rhs=b2g_r, start=False, stop=True)
                nc.tensor.matmul(out=msg0B[:, osl], lhsT=mT[:, osl],
                                 rhs=msgW[:], start=True, stop=biases_triv)
                if not biases_triv:
                    nc.tensor.matmul(out=msg0B[:, osl], lhsT=ones1[:],
                                     rhs=msgb_r, start=False, stop=True)
            sgB = wpool.tile([P, B], BF16, tag="sgB")
            nc.scalar.activation(out=sgB[:], in_=gateB[:], func=AF.Sigmoid,
                                 bias=0.0, scale=1.0)

            for t in range(CPB):
                ci = e0 // P + t
                osl = slice(t * P, (t + 1) * P)
                msgP = mpool.tile([P, P], BF16, tag="msgP")
                nc.vector.tensor_tensor(out=msgP[:], in0=msg0B[:, osl],
                                        in1=sgB[:, osl], op=OP.mult)
                s_t = mpool.tile([P, P], BF16, tag="S")
                nc.vector.tensor_scalar(out=s_t[:], in0=iota_b[:],
                                        scalar1=dcols_t[:, ci:ci + 1],
                                        scalar2=None, op0=OP.is_equal)
                dt = chunk_tile[ci]
                if tile_first[dt] == ci:
                    agg_new = psG.tile([P, P], F32, space="PSUM", tag="aggr")
                    aggr_tiles[dt] = agg_new
                nc.tensor.matmul(out=aggr_tiles[dt][:], lhsT=s_t[:],
                                 rhs=msgP[:], start=(tile_first[dt] == ci),
                                 stop=False)
                if tile_last[dt] == ci:
                    out_stage(dt)
    nc.compile()
    return nc


def _prep_phase1(x, node_time, nn, gn):
    xpad = np.zeros((NPAD, ND), np.float32)
    xpad[:N] = x
    ntpad = np.zeros((NPAD,), np.float32)
    ntpad[:N] = node_time[:, 0]
    xT_all = _bf16(xpad.T)
    n1_all = _bf16(np.stack([ntpad, np.ones_like(ntpad)]))

    W1n_c = _center(nn['W1'])
    b1n_c = nn['b1'] - nn['b1'].mean()
    Wg_c = _center(gn['W1'])
    b1g_c = gn['b1'] - gn['b1'].mean()
    Wxg_c = Wg_c[ED:ED + ND]
    wtg_c = Wg_c[ED + ND]

    brows1 = _bf16(np.concatenate([b1n_c, nn['b2'], wtg_c])[None, :])
    gbc = np.zeros((P, 2 * P), np.float32)
    gbc[:, :P] = np.broadcast_to(nn['g1'], (P, P))
    gbc[:, P:] = np.broadcast_to(nn['be1'], (P, P))
    wts1 = _bf16(np.concatenate([W1n_c, nn['W2'], Wxg_c, gbc], axis=1))

    in_maps1 = []
    for c in range(NCORES):
        cs = slice(c * TILES_PC * P, (c + 1) * TILES_PC * P)
        in_maps1.append({
            "xT": np.ascontiguousarray(xT_all[:, cs]),
            "n1T": np.ascontiguousarray(n1_all[:, cs]),
            "wts": wts1,
            "brows": brows1,
        })
    return in_maps1, xT_all, Wg_c, b1g_c


def _prep_phase2(row, col, edge_attr, tbl, xT_all, en, gn, pp, Wg_c, b1g_c):
    perm = np.argsort(row, kind='stable')
    row_s = row[perm]
    col_s = col[perm]
    ea_s = edge_attr[perm]

    bounds = np.searchsorted(row_s, np.arange(0, NPAD + 1, P))
    tile_cnt = (bounds[1:] - bounds[:-1]).astype(np.int64)
    tile_chunks_each = np.maximum((tile_cnt + P - 1) // P, 1)
    shared = tile_chunks_each.reshape(NCORES, TILES_PC).max(axis=0)
    cmax = int(shared.sum())
    pad_to = ((cmax + CPB - 1) // CPB) * CPB
    shared[-1] += pad_to - cmax
    cmax = pad_to
    shared_list = [int(v) for v in shared]

    in_maps2 = []
    for c in range(NCORES):
        segs_e, segs_cols, segs_dest = [], [], []
        for j in range(TILES_PC):
            t = c * TILES_PC + j
            lo, hi = bounds[t], bounds[t + 1]
            n = hi - lo
            npad_t = shared_list[j] * P
            eat = np.zeros((npad_t, ED), np.float32)
            eat[:n] = ea_s[lo:hi]
            ct = np.zeros((npad_t,), np.int64)
            ct[:n] = col_s[lo:hi]
            dcol = np.full((npad_t,), -1.0, np.float32)
            dcol[:n] = (row_s[lo:hi] - t * P).astype(np.float32)
            segs_e.append(eat)
            segs_cols.append(ct)
            segs_dest.append(dcol)
        e_all = np.concatenate(segs_e)
        c_all = np.concatenate(segs_cols)
        d_all = np.concatenate(segs_dest)
        eT_aug = np.ones((ED + 1, cmax * P), np.float32)
        eT_aug[:ED] = e_all.T
        hnu = tbl[c_all]
        in_maps2.append({
            "eT": _bf16(eT_aug),
            "hnuT": np.ascontiguousarray(hnu.T),
            "dcols": np.ascontiguousarray(
                d_all.reshape(cmax, P).T.astype(np.float32)),
            "xT2": np.ascontiguousarray(
                xT_all[:, c * TILES_PC * P:(c + 1) * TILES_PC * P]),
        })

    W1e_c = _center(en['W1'])
    b1e_c = en['b1'] - en['b1'].mean()
    Weg_c = Wg_c[:ED]
    wA_np = np.zeros((ED + 1, 2 * P), np.float32)
    wA_np[:ED, :P] = W1e_c
    wA_np[ED, :P] = b1e_c
    wA_np[:ED, P:] = Weg_c
    wA_np[ED, P:] = b1g_c

    miscr_np = _bf16(np.concatenate([
        pp['msg_b'], gn['b2'], pp['cen_b'], pp['out_b'],
        pp['ln_g'], pp['ln_b']])[None, :])
    wB_np = np.concatenate([en['W2'], gn['W2'], pp['msg_W'], pp['cen_W'],
                            pp['out_W']], axis=1)
    bcol_np = np.zeros((P, 4), np.float32)
    bcol_np[:, 0] = en['b2']
    bcol_np[:, 1] = en['g1']
    bcol_np[:, 2] = en['be1']
    gcolg_np = np.zeros((P, 2), np.float32)
    gcolg_np[:, 0] = gn['g1']
    gcolg_np[:, 1] = gn['be1']

    wA_b = _bf16(wA_np)
    wB_b = _bf16(wB_np)
    for m in in_maps2:
        m["wA"] = wA_b
        m["wB"] = wB_b
        m["miscr"] = miscr_np
        m["bcol"] = bcol_np
        m["gcolg"] = gcolg_np
    return in_maps2, cmax, shared_list


def kernel(x, edge_index, edge_attr, node_time, node_net, edge_net, gate_net,
           params):
    import sys, os
    if '/opt/trn_rl_repo' not in sys.path:
        sys.path.insert(0, '/opt/trn_rl_repo')
    from concourse.bass_utils import run_bass_kernel_spmd

    x = _to_np(x)
    edge_attr = _to_np(edge_attr)
    node_time = _to_np(node_time)
    ei = np.asarray(edge_index)
    row = ei[0].astype(np.int64)
    col = ei[1].astype(np.int64)
    nn = {k: _to_np(v) for k, v in node_net.items()}
    en = {k: _to_np(v) for k, v in edge_net.items()}
    gn = {k: _to_np(v) for k, v in gate_net.items()}
    pp = {k: _to_np(v) for k, v in params.items()}

    affine_triv_n = _trivial(nn['g1'], nn['be1'])
    affine_triv_e = _trivial(en['g1'], en['be1'])
    affine_triv_g = _trivial(gn['g1'], gn['be1'])
    affine_triv_o = _trivial(pp['ln_g'], pp['ln_b'])
    biases_triv = bool(np.all(pp['msg_b'] == 0) and np.all(gn['b2'] == 0))

    in_maps1, xT_all, Wg_c, b1g_c = _prep_phase1(x, node_time, nn, gn)
    nc1 = _build_phase1(affine_triv_n)
    res1 = run_bass_kernel_spmd(nc1, in_maps1, list(range(NCORES)))
    tbl = np.concatenate([res1.results[c]["tbl"] for c in range(NCORES)],
                         axis=0)

    in_maps2, cmax, shared_list = _prep_phase2(
        row, col, edge_attr, tbl, xT_all, en, gn, pp, Wg_c, b1g_c)
    nc2 = _build_phase2(cmax, shared_list, affine_triv_e, affine_triv_g,
                        affine_triv_o, biases_triv)
    res2 = run_bass_kernel_spmd(nc2, in_maps2, list(range(NCORES)))
    globals()['LAST_BUILDS'] = (nc1, in_maps1, nc2, in_maps2)
    out = np.concatenate([res2.results[c]["out"] for c in range(NCORES)],
                         axis=0)
    return out[:N].astype(np.float32)
